# revision 2
# baseline (speedup 1.0000x reference)
"""MiniSTU Trainium2 kernel.

Reformulation (no FFT): the reference computes, per batch b,
    out = sum_k T_k @ (x @ Mp_k)  +  sgn ⊙ sum_k T_k @ (sgn ⊙ (x @ Mm_k))
where T_k is the lower-triangular Toeplitz matrix T_k[l,s] = phi[l-s,k]
and sgn[l] = (-1)^l.  (The sign-flipped filter Toeplitz satisfies
T~ = S T S with S = diag(sgn), which lets both branches share one set of
Toeplitz blocks.)

On device (per core; 8 cores = batch(2) x output-quarter(4), no collectives):
  stage 1:  A[l, (k,s,o)] = x_b^T-chunks @ M-matrix   (contraction D=512)
            with the minus branch sign-fixed on the PSUM->SBUF copy.
  stage 2:  out[I] = sum_{J<=I, k} PH[I-J,k]^T @ A[J, k-slice]
            (128x128 Toeplitz blocks of phi as stationary operands,
             free dim 256 = o-slice(128) x sign(2)).
Run twice over k-halves (8 filters each) to fit SBUF; fp16 operands,
fp32 PSUM accumulation.
"""

import numpy as np

B, L, D, O, K, P = 2, 2048, 512, 512, 16, 128
NB = L // P       # 16 l-blocks
KH = 2            # k halves
KPH = K // KH     # 8 filters per half
NOQ = 4           # o-quarters
OS = O // NOQ     # 128 per-core o slice
N_CORES = 8

_cache = {}


def _build_bass():
    import concourse.mybir as mybir
    import concourse.tile as tile
    from concourse import bacc

    dt = mybir.dt
    f16, f32 = dt.float16, dt.float32

    nc = bacc.Bacc("TRN2", target_bir_lowering=False, debug=False,
                   num_devices=N_CORES)

    # DRAM layouts exactly match the SBUF tiles (host does all permutes).
    xt_d = nc.dram_tensor("xt", [P, 4, L], f16, kind="ExternalInput")
    mx_d = nc.dram_tensor("mx", [P, 4, K * 2 * OS], f16, kind="ExternalInput")
    ph_d = nc.dram_tensor("ph", [KH, P, NB * KPH * P], f16, kind="ExternalInput")
    sg_d = nc.dram_tensor("sg", [P, 1], f32, kind="ExternalInput")
    out_d = nc.dram_tensor("out", [P, NB * OS], f32, kind="ExternalOutput")

    CH = KPH * 2 * OS          # 2048 columns per k-half in mx/a
    with tile.TileContext(nc) as tc:
        with (
            tc.tile_pool(name="const", bufs=1) as cpool,
            tc.tile_pool(name="phpool", bufs=1) as phpool,
            tc.tile_pool(name="apool", bufs=1) as apool,
            tc.tile_pool(name="opool", bufs=1) as opool,
            tc.tile_pool(name="ps1", bufs=3, space="PSUM") as ps1pool,
            tc.tile_pool(name="ps2", bufs=2, space="PSUM") as ps2pool,
        ):
            xt = cpool.tile([P, 4, L], f16, tag="xt")
            mx = cpool.tile([P, 4, K * 2 * OS], f16, tag="mx")
            sg = cpool.tile([P, 1], f32, tag="sg")
            a_sb = apool.tile([P, NB, CH], f16, tag="a")
            outacc = opool.tile([P, NB, 2 * OS], f32, tag="outacc")
            res = opool.tile([P, NB, OS], f32, tag="res")

            nc.sync.dma_start(out=xt[:], in_=xt_d[:])
            nc.sync.dma_start(out=mx[:], in_=mx_d[:])
            nc.sync.dma_start(out=sg[:], in_=sg_d[:])

            for kh in range(KH):
                ph = phpool.tile([P, NB * KPH * P], f16, tag="ph")
                nc.sync.dma_start(out=ph[:], in_=ph_d[kh])

                # ---- stage 1: A[lt] = sum_dc xt[dc,lt]^T.T @ mx[dc, half]
                for lt in range(NB):
                    for t in range(2):             # two psum tiles of 1024 cols
                        ps = ps1pool.tile([P, 1024], f32, tag="ps1")
                        for dc in range(4):
                            for n2 in range(2):    # 512-wide matmuls
                                nc.tensor.matmul(
                                    ps[:, n2 * 512:(n2 + 1) * 512],
                                    xt[:, dc, lt * P:(lt + 1) * P],
                                    mx[:, dc, kh * CH + t * 1024 + n2 * 512:
                                       kh * CH + t * 1024 + (n2 + 1) * 512],
                                    start=(dc == 0), stop=(dc == 3),
                                )
                        # copy to A (fp16), applying sgn to the minus columns
                        psv = ps[:].rearrange("p (a s o) -> p a s o", a=4, s=2, o=OS)
                        av = a_sb[:, lt, t * 1024:(t + 1) * 1024].rearrange(
                            "p (a s o) -> p a s o", a=4, s=2, o=OS)
                        nc.vector.tensor_copy(av[:, :, 0, :], psv[:, :, 0, :])
                        nc.vector.tensor_scalar_mul(
                            av[:, :, 1, :], psv[:, :, 1, :], sg[:])

                # ---- stage 2: out[I] += sum_{J<=I,k} PH[I-J,k].T @ A[J,k]
                for I in range(NB):
                    ps2 = ps2pool.tile([P, 2 * OS], f32, tag="ps2")
                    n_mm = 8 * (I + 1)
                    i_mm = 0
                    for J in range(I, -1, -1):     # d = I-J ascending in reuse
                        d = I - J
                        for kl in range(KPH):
                            nc.tensor.matmul(
                                ps2[:],
                                ph[:, (d * KPH + kl) * P:(d * KPH + kl + 1) * P],
                                a_sb[:, J, kl * 2 * OS:(kl + 1) * 2 * OS],
                                start=(i_mm == 0), stop=(i_mm == n_mm - 1),
                            )
                            i_mm += 1
                    if kh == 0:
                        nc.vector.tensor_copy(outacc[:, I, :], ps2[:])
                    else:
                        nc.vector.tensor_add(outacc[:, I, :], outacc[:, I, :], ps2[:])

            # ---- final: res = plus + sgn * minus
            ov = outacc[:].rearrange("p i (s o) -> p i s o", s=2, o=OS)
            nc.vector.scalar_tensor_tensor(
                res[:], ov[:, :, 1, :], sg[:], ov[:, :, 0, :],
                op0=mybir.AluOpType.mult, op1=mybir.AluOpType.add,
            )
            nc.sync.dma_start(out=out_d[:], in_=res[:].rearrange("p i o -> p (i o)"))

    nc.compile()
    return nc


def _prep_inputs(x, phi, M_phi_plus, M_phi_minus):
    """Host-side shard prep. Returns list of 8 input dicts (cores = b*4 + oq)."""
    sgn = np.where(np.arange(L) % 2 == 1, -1.0, 1.0).astype(np.float32)

    # xt[p, dc, l] = x[b, l, dc*128+p]
    xts = []
    for b in range(B):
        xt = np.ascontiguousarray(
            x[b].T.reshape(4, P, L).transpose(1, 0, 2)).astype(np.float16)
        xts.append(xt)

    # mx[p, dc, k*256 + s*128 + oo] = M_s[k, dc*128+p, oq*128+oo]
    mcat = np.stack([M_phi_plus, M_phi_minus], axis=1)  # [K, 2, D, O]
    mxs = []
    for oq in range(NOQ):
        m = mcat[:, :, :, oq * OS:(oq + 1) * OS]        # [K, 2, D, OS]
        m = m.transpose(2, 0, 1, 3).reshape(D, K * 2 * OS)  # [D, K*2*OS]
        mx = np.ascontiguousarray(
            m.reshape(4, P, K * 2 * OS).transpose(1, 0, 2)).astype(np.float16)
        mxs.append(mx)

    # ph[kh, pp, (d*KPH + kl)*P + p] = phi[d*P + p - pp, kh*KPH+kl]
    idx = np.arange(P)
    diff = idx[None, :] - idx[:, None]                  # [pp, p] = p - pp
    v = np.arange(NB)[:, None, None] * P + diff[None]   # [d, pp, p]
    valid = v >= 0
    phb = np.zeros((NB, P, P, K), dtype=np.float32)     # [d, pp, p, k]
    phb[valid] = phi[v[valid], :]
    # -> [kh, pp, d, kl, p]
    phb = phb.reshape(NB, P, P, KH, KPH).transpose(3, 1, 0, 4, 2)
    ph = np.ascontiguousarray(phb.reshape(KH, P, NB * KPH * P)).astype(np.float16)

    sg = np.ascontiguousarray(sgn[:P].reshape(P, 1))

    in_maps = []
    for b in range(B):
        for oq in range(NOQ):
            in_maps.append({"xt": xts[b], "mx": mxs[oq], "ph": ph, "sg": sg})
    return in_maps


def kernel(x, phi, M_phi_plus, M_phi_minus):
    from concourse.bass_utils import run_bass_kernel_spmd

    x = np.asarray(x, dtype=np.float32)
    phi = np.asarray(phi, dtype=np.float32)
    M_phi_plus = np.asarray(M_phi_plus, dtype=np.float32)
    M_phi_minus = np.asarray(M_phi_minus, dtype=np.float32)

    if "nc" not in _cache:
        _cache["nc"] = _build_bass()
    nc = _cache["nc"]

    in_maps = _prep_inputs(x, phi, M_phi_plus, M_phi_minus)
    results = run_bass_kernel_spmd(nc, in_maps, core_ids=list(range(N_CORES)))

    out = np.empty((B, L, O), dtype=np.float32)
    for c in range(N_CORES):
        b, oq = divmod(c, NOQ)
        r = results.results[c]["out"]                   # [P, NB*OS]
        blk = r.reshape(P, NB, OS).transpose(1, 0, 2).reshape(L, OS)
        out[b, :, oq * OS:(oq + 1) * OS] = blk
    return out


# revision 6
# speedup vs baseline: 271.8695x; 271.8695x over previous
"""MiniSTU Trainium2 kernel.

Reformulation (no FFT): the reference computes, per batch b,
    out = sum_k T_k @ (x @ Mp_k)  +  sgn ⊙ sum_k T_k @ (sgn ⊙ (x @ Mm_k))
where T_k is the lower-triangular Toeplitz matrix T_k[l,s] = phi[l-s,k]
and sgn[l] = (-1)^l.  (The sign-flipped filter Toeplitz satisfies
T~ = S T S with S = diag(sgn), which lets both branches share one set of
Toeplitz blocks.)

On device (per core; 8 cores = batch(2) x output-quarter(4), no collectives):
  stage 1:  A[l, (k,s,o)] = x_b^T-chunks @ M-matrix   (contraction D=512)
            with the minus branch sign-fixed on the PSUM->SBUF copy.
  stage 2:  out[I] = sum_{J<=I, k} PH[I-J,k]^T @ A[J, k-slice]
            (128x128 Toeplitz blocks of phi as stationary operands,
             free dim 256 = o-slice(128) x sign(2)).
Run twice over k-halves (8 filters each) to fit SBUF; fp16 operands,
fp32 PSUM accumulation.
"""

import numpy as np

B, L, D, O, K, P = 2, 2048, 512, 512, 16, 128
NB = L // P       # 16 l-blocks
KH = 2            # k halves
KPH = K // KH     # 8 filters per half
NOQ = 4           # o-quarters
OS = O // NOQ     # 128 per-core o slice
N_CORES = 8

_cache = {}


def _build_bass(reps=1):
    import contextlib
    import concourse.mybir as mybir
    import concourse.tile as tile
    from concourse import bacc

    dt = mybir.dt
    f16, f32 = dt.float16, dt.float32

    nc = bacc.Bacc("TRN2", target_bir_lowering=False, debug=False,
                   num_devices=N_CORES)

    # DRAM layouts exactly match the SBUF tiles (host does all permutes).
    xt_d = nc.dram_tensor("xt", [P, 4, L], f16, kind="ExternalInput")
    mx_d = nc.dram_tensor("mx", [P, 4, K * 2 * OS], f16, kind="ExternalInput")
    ph_d = nc.dram_tensor("ph", [KH, P, NB * KPH * P], f16, kind="ExternalInput")
    sg_d = nc.dram_tensor("sg", [P, 1], f32, kind="ExternalInput")
    out_d = nc.dram_tensor("out", [P, NB * OS], f32, kind="ExternalOutput")

    CH = KPH * 2 * OS          # 2048 columns per k-half in mx/a
    with tile.TileContext(nc) as tc:
        with (
            tc.tile_pool(name="const", bufs=1) as cpool,
            tc.tile_pool(name="phpool", bufs=1) as phpool,
            tc.tile_pool(name="apool", bufs=1) as apool,
            tc.tile_pool(name="opool", bufs=1) as opool,
            tc.tile_pool(name="ps1", bufs=3, space="PSUM") as ps1pool,
            tc.tile_pool(name="ps2", bufs=2, space="PSUM") as ps2pool,
        ):
            xt = cpool.tile([P, 4, L], f16, tag="xt")
            mx = cpool.tile([P, 4, K * 2 * OS], f16, tag="mx")
            sg = cpool.tile([P, 1], f32, tag="sg")

            nc.sync.dma_start(out=xt[:], in_=xt_d[:])
            nc.sync.dma_start(out=mx[:], in_=mx_d[:])
            nc.sync.dma_start(out=sg[:], in_=sg_d[:])

            loop_cm = tc.For_i(0, reps, 1) if reps > 1 else contextlib.nullcontext()
            with loop_cm:
                _emit_body(nc, tc, mybir, f16, f32, xt, mx, sg, ph_d, phpool,
                           apool, opool, ps1pool, ps2pool, out_d)

    nc.compile()
    return nc


def _emit_body(nc, tc, mybir, f16, f32, xt, mx, sg, ph_d, phpool,
               apool, opool, ps1pool, ps2pool, out_d):
    CH = KPH * 2 * OS
    if True:
        if True:
            a_sb = apool.tile([P, NB, CH], f16, tag="a")
            outacc = opool.tile([P, NB, 2 * OS], f32, tag="outacc")
            res = opool.tile([P, NB, OS], f32, tag="res")

            for kh in range(KH):
                ph = phpool.tile([P, NB * KPH * P], f16, tag="ph")
                nc.sync.dma_start(out=ph[:], in_=ph_d[kh])

                # ---- stage 1: A[lt] = sum_dc xt[dc,lt]^T.T @ mx[dc, half]
                for lt in range(NB):
                    for t in range(2):             # two psum tiles of 1024 cols
                        ps = ps1pool.tile([P, 1024], f32, tag="ps1")
                        for dc in range(4):
                            for n2 in range(2):    # 512-wide matmuls
                                nc.tensor.matmul(
                                    ps[:, n2 * 512:(n2 + 1) * 512],
                                    xt[:, dc, lt * P:(lt + 1) * P],
                                    mx[:, dc, kh * CH + t * 1024 + n2 * 512:
                                       kh * CH + t * 1024 + (n2 + 1) * 512],
                                    start=(dc == 0), stop=(dc == 3),
                                )
                        # copy to A (fp16), applying sgn to the minus columns
                        psv = ps[:].rearrange("p (a s o) -> p a s o", a=4, s=2, o=OS)
                        av = a_sb[:, lt, t * 1024:(t + 1) * 1024].rearrange(
                            "p (a s o) -> p a s o", a=4, s=2, o=OS)
                        nc.vector.tensor_copy(av[:, :, 0, :], psv[:, :, 0, :])
                        nc.vector.tensor_scalar_mul(
                            av[:, :, 1, :], psv[:, :, 1, :], sg[:])

                # ---- stage 2: out[I] += sum_{J<=I,k} PH[I-J,k].T @ A[J,k]
                for I in range(NB):
                    ps2 = ps2pool.tile([P, 2 * OS], f32, tag="ps2")
                    n_mm = 8 * (I + 1)
                    i_mm = 0
                    for J in range(I, -1, -1):     # d = I-J ascending in reuse
                        d = I - J
                        for kl in range(KPH):
                            nc.tensor.matmul(
                                ps2[:],
                                ph[:, (d * KPH + kl) * P:(d * KPH + kl + 1) * P],
                                a_sb[:, J, kl * 2 * OS:(kl + 1) * 2 * OS],
                                start=(i_mm == 0), stop=(i_mm == n_mm - 1),
                            )
                            i_mm += 1
                    if kh == 0:
                        nc.vector.tensor_copy(outacc[:, I, :], ps2[:])
                    else:
                        nc.vector.tensor_add(outacc[:, I, :], outacc[:, I, :], ps2[:])

            # ---- final: res = plus + sgn * minus
            ov = outacc[:].rearrange("p i (s o) -> p i s o", s=2, o=OS)
            nc.vector.scalar_tensor_tensor(
                res[:], ov[:, :, 1, :], sg[:], ov[:, :, 0, :],
                op0=mybir.AluOpType.mult, op1=mybir.AluOpType.add,
            )
            nc.sync.dma_start(out=out_d[:], in_=res[:].rearrange("p i o -> p (i o)"))


def _prep_inputs(x, phi, M_phi_plus, M_phi_minus):
    """Host-side shard prep. Returns list of 8 input dicts (cores = b*4 + oq)."""
    sgn = np.where(np.arange(L) % 2 == 1, -1.0, 1.0).astype(np.float32)

    # xt[p, dc, l] = x[b, l, dc*128+p]
    xts = []
    for b in range(B):
        xt = np.ascontiguousarray(
            x[b].T.reshape(4, P, L).transpose(1, 0, 2)).astype(np.float16)
        xts.append(xt)

    # mx[p, dc, k*256 + s*128 + oo] = M_s[k, dc*128+p, oq*128+oo]
    mcat = np.stack([M_phi_plus, M_phi_minus], axis=1)  # [K, 2, D, O]
    mxs = []
    for oq in range(NOQ):
        m = mcat[:, :, :, oq * OS:(oq + 1) * OS]        # [K, 2, D, OS]
        m = m.transpose(2, 0, 1, 3).reshape(D, K * 2 * OS)  # [D, K*2*OS]
        mx = np.ascontiguousarray(
            m.reshape(4, P, K * 2 * OS).transpose(1, 0, 2)).astype(np.float16)
        mxs.append(mx)

    # ph[kh, pp, (d*KPH + kl)*P + p] = phi[d*P + p - pp, kh*KPH+kl]
    idx = np.arange(P)
    diff = idx[None, :] - idx[:, None]                  # [pp, p] = p - pp
    v = np.arange(NB)[:, None, None] * P + diff[None]   # [d, pp, p]
    valid = v >= 0
    phb = np.zeros((NB, P, P, K), dtype=np.float32)     # [d, pp, p, k]
    phb[valid] = phi[v[valid], :]
    # -> [kh, pp, d, kl, p]
    phb = phb.reshape(NB, P, P, KH, KPH).transpose(3, 1, 0, 4, 2)
    ph = np.ascontiguousarray(phb.reshape(KH, P, NB * KPH * P)).astype(np.float16)

    sg = np.ascontiguousarray(sgn[:P].reshape(P, 1))

    in_maps = []
    for b in range(B):
        for oq in range(NOQ):
            in_maps.append({"xt": xts[b], "mx": mxs[oq], "ph": ph, "sg": sg})
    return in_maps


def kernel(x, phi, M_phi_plus, M_phi_minus):
    from concourse.bass_utils import run_bass_kernel_spmd

    x = np.asarray(x, dtype=np.float32)
    phi = np.asarray(phi, dtype=np.float32)
    M_phi_plus = np.asarray(M_phi_plus, dtype=np.float32)
    M_phi_minus = np.asarray(M_phi_minus, dtype=np.float32)

    if "nc" not in _cache:
        _cache["nc"] = _build_bass()
    nc = _cache["nc"]

    in_maps = _prep_inputs(x, phi, M_phi_plus, M_phi_minus)
    results = run_bass_kernel_spmd(nc, in_maps, core_ids=list(range(N_CORES)))

    out = np.empty((B, L, O), dtype=np.float32)
    for c in range(N_CORES):
        b, oq = divmod(c, NOQ)
        r = results.results[c]["out"]                   # [P, NB*OS]
        blk = r.reshape(P, NB, OS).transpose(1, 0, 2).reshape(L, OS)
        out[b, :, oq * OS:(oq + 1) * OS] = blk
    return out


# revision 8
# speedup vs baseline: 313.0115x; 1.1513x over previous
"""MiniSTU Trainium2 kernel.

Reformulation (no FFT): the reference computes, per batch b,
    out = sum_k T_k @ (x @ Mp_k)  +  sgn ⊙ sum_k T_k @ (sgn ⊙ (x @ Mm_k))
where T_k is the lower-triangular Toeplitz matrix T_k[l,s] = phi[l-s,k]
and sgn[l] = (-1)^l.  (The sign-flipped filter Toeplitz satisfies
T~ = S T S with S = diag(sgn), which lets both branches share one set of
Toeplitz blocks.)

On device (per core; 8 cores = batch(2) x output-quarter(4), no collectives):
  stage 1:  A[l, (k,s,o)] = x_b^T-chunks @ M-matrix   (contraction D=512)
            with the minus branch sign-fixed on the PSUM->SBUF copy.
  stage 2:  out[I] = sum_{J<=I, k} PH[I-J,k]^T @ A[J, k-slice]
            (128x128 Toeplitz blocks of phi as stationary operands,
             free dim 256 = o-slice(128) x sign(2)).
Run twice over k-halves (8 filters each) to fit SBUF; fp16 operands,
fp32 PSUM accumulation.
"""

import numpy as np

B, L, D, O, K, P = 2, 2048, 512, 512, 16, 128
NB = L // P       # 16 l-blocks
KH = 2            # k halves
KPH = K // KH     # 8 filters per half
NOQ = 4           # o-quarters
OS = O // NOQ     # 128 per-core o slice
N_CORES = 8

_cache = {}


def _build_bass(reps=1):
    import contextlib
    import concourse.mybir as mybir
    import concourse.tile as tile
    from concourse import bacc

    dt = mybir.dt
    f16, f32 = dt.float16, dt.float32

    nc = bacc.Bacc("TRN2", target_bir_lowering=False, debug=False,
                   num_devices=N_CORES)

    # DRAM layouts exactly match the SBUF tiles (host does all permutes).
    xt_d = nc.dram_tensor("xt", [P, 4, L], f16, kind="ExternalInput")
    mx_d = nc.dram_tensor("mx", [P, 4, K * 2 * OS], f16, kind="ExternalInput")
    ph_d = nc.dram_tensor("ph", [KH, P, NB * KPH * P], f16, kind="ExternalInput")
    sg_d = nc.dram_tensor("sg", [P, 1], f32, kind="ExternalInput")
    out_d = nc.dram_tensor("out", [P, NB * OS], f32, kind="ExternalOutput")

    CH = KPH * 2 * OS          # 2048 columns per k-half in mx/a
    with tile.TileContext(nc) as tc:
        with (
            tc.tile_pool(name="const", bufs=1) as cpool,
            tc.tile_pool(name="phpool", bufs=1) as phpool,
            tc.tile_pool(name="apool", bufs=1) as apool,
            tc.tile_pool(name="opool", bufs=1) as opool,
        ):
            xt = cpool.tile([P, 4, L], f16, tag="xt")
            mx = cpool.tile([P, 4, K * 2 * OS], f16, tag="mx")
            sg = cpool.tile([P, 1], f32, tag="sg")

            nc.sync.dma_start(out=xt[:], in_=xt_d[:])
            nc.sync.dma_start(out=mx[:], in_=mx_d[:])
            nc.sync.dma_start(out=sg[:], in_=sg_d[:])

            loop_cm = tc.For_i(0, reps, 1) if reps > 1 else contextlib.nullcontext()
            with loop_cm:
                _emit_body(nc, tc, mybir, f16, f32, xt, mx, sg, ph_d, phpool,
                           apool, opool, out_d)

    nc.compile()
    return nc


def _emit_body(nc, tc, mybir, f16, f32, xt, mx, sg, ph_d, phpool,
               apool, opool, out_d):
    CH = KPH * 2 * OS
    a_sb = apool.tile([P, NB, CH], f16, tag="a")
    outacc = opool.tile([P, NB, 2 * OS], f32, tag="outacc")
    res = opool.tile([P, NB, OS], f32, tag="res")

    for kh in range(KH):
        ph = phpool.tile([P, NB * KPH * P], f16, tag="ph")
        nc.sync.dma_start(out=ph[:], in_=ph_d[kh])

        # ---- stage 1: A[lt] = sum_dc xt[dc,lt]^T.T @ mx[dc, half]
        # dc-outer so each xt stationary block serves 4 matmuls; each
        # 512-col psum region is one full bank (interleaved groups are
        # safe only at bank granularity: start=True clears whole-bank
        # has_written bits).
        with tc.tile_pool(name="ps1", bufs=2, space="PSUM") as ps1pool:
            for lt in range(NB):
                ps = ps1pool.tile([P, 2048], f32, tag="ps1")
                for dc in range(4):
                    for n in range(4):
                        nc.tensor.matmul(
                            ps[:, n * 512:(n + 1) * 512],
                            xt[:, dc, lt * P:(lt + 1) * P],
                            mx[:, dc, kh * CH + n * 512:kh * CH + (n + 1) * 512],
                            start=(dc == 0), stop=(dc == 3),
                        )
                # copy to A (fp16), applying sgn to the minus columns
                psv = ps[:].rearrange("p (a s o) -> p a s o", a=KPH, s=2, o=OS)
                av = a_sb[:, lt, :].rearrange(
                    "p (a s o) -> p a s o", a=KPH, s=2, o=OS)
                nc.vector.tensor_copy(av[:, :, 0, :], psv[:, :, 0, :])
                nc.vector.tensor_scalar_mul(av[:, :, 1, :], psv[:, :, 1, :], sg[:])

        # ---- stage 2: out[I] += sum_{J<=I,k} PH[I-J,k].T @ A[J,k]
        # (d,kl)-outer: one weight load per Toeplitz block, up to 16
        # matmuls each.  8 accumulators at a time, one PSUM bank each
        # (half used) so interleaved accumulation groups never share a
        # bank.  Two I-halves per k-half.
        with tc.tile_pool(name="ps2", bufs=1, space="PSUM") as ps2pool:
            for ih in range(2):
                i_lo = ih * 8
                ps2 = ps2pool.tile([P, 8, 512], f32, tag="ps2")
                for d in range(NB):
                    j_lo = max(0, i_lo - d)
                    j_hi = min(NB, i_lo + 8 - d)   # J range with I=J+d in half
                    if j_hi <= j_lo:
                        continue
                    for kl in range(KPH):
                        for J in range(j_lo, j_hi):
                            I = J + d
                            nc.tensor.matmul(
                                ps2[:, I - i_lo, 0:2 * OS],
                                ph[:, (d * KPH + kl) * P:(d * KPH + kl + 1) * P],
                                a_sb[:, J, kl * 2 * OS:(kl + 1) * 2 * OS],
                                start=(d == 0 and kl == 0),
                                stop=(d == I and kl == KPH - 1),
                            )
                    # accumulator I == d closes after its (d == I) pass
                    if i_lo <= d < i_lo + 8:
                        I = d
                        if kh == 0:
                            nc.vector.tensor_copy(
                                outacc[:, I, :], ps2[:, I - i_lo, 0:2 * OS])
                        else:
                            nc.vector.tensor_add(
                                outacc[:, I, :], outacc[:, I, :],
                                ps2[:, I - i_lo, 0:2 * OS])

    # ---- final: res = plus + sgn * minus
    ov = outacc[:].rearrange("p i (s o) -> p i s o", s=2, o=OS)
    nc.vector.scalar_tensor_tensor(
        res[:], ov[:, :, 1, :], sg[:], ov[:, :, 0, :],
        op0=mybir.AluOpType.mult, op1=mybir.AluOpType.add,
    )
    nc.sync.dma_start(out=out_d[:], in_=res[:].rearrange("p i o -> p (i o)"))


def _prep_inputs(x, phi, M_phi_plus, M_phi_minus):
    """Host-side shard prep. Returns list of 8 input dicts (cores = b*4 + oq)."""
    sgn = np.where(np.arange(L) % 2 == 1, -1.0, 1.0).astype(np.float32)

    # xt[p, dc, l] = x[b, l, dc*128+p]
    xts = []
    for b in range(B):
        xt = np.ascontiguousarray(
            x[b].T.reshape(4, P, L).transpose(1, 0, 2)).astype(np.float16)
        xts.append(xt)

    # mx[p, dc, k*256 + s*128 + oo] = M_s[k, dc*128+p, oq*128+oo]
    mcat = np.stack([M_phi_plus, M_phi_minus], axis=1)  # [K, 2, D, O]
    mxs = []
    for oq in range(NOQ):
        m = mcat[:, :, :, oq * OS:(oq + 1) * OS]        # [K, 2, D, OS]
        m = m.transpose(2, 0, 1, 3).reshape(D, K * 2 * OS)  # [D, K*2*OS]
        mx = np.ascontiguousarray(
            m.reshape(4, P, K * 2 * OS).transpose(1, 0, 2)).astype(np.float16)
        mxs.append(mx)

    # ph[kh, pp, (d*KPH + kl)*P + p] = phi[d*P + p - pp, kh*KPH+kl]
    idx = np.arange(P)
    diff = idx[None, :] - idx[:, None]                  # [pp, p] = p - pp
    v = np.arange(NB)[:, None, None] * P + diff[None]   # [d, pp, p]
    valid = v >= 0
    phb = np.zeros((NB, P, P, K), dtype=np.float32)     # [d, pp, p, k]
    phb[valid] = phi[v[valid], :]
    # -> [kh, pp, d, kl, p]
    phb = phb.reshape(NB, P, P, KH, KPH).transpose(3, 1, 0, 4, 2)
    ph = np.ascontiguousarray(phb.reshape(KH, P, NB * KPH * P)).astype(np.float16)

    sg = np.ascontiguousarray(sgn[:P].reshape(P, 1))

    in_maps = []
    for b in range(B):
        for oq in range(NOQ):
            in_maps.append({"xt": xts[b], "mx": mxs[oq], "ph": ph, "sg": sg})
    return in_maps


def kernel(x, phi, M_phi_plus, M_phi_minus):
    from concourse.bass_utils import run_bass_kernel_spmd

    x = np.asarray(x, dtype=np.float32)
    phi = np.asarray(phi, dtype=np.float32)
    M_phi_plus = np.asarray(M_phi_plus, dtype=np.float32)
    M_phi_minus = np.asarray(M_phi_minus, dtype=np.float32)

    if "nc" not in _cache:
        _cache["nc"] = _build_bass()
    nc = _cache["nc"]

    in_maps = _prep_inputs(x, phi, M_phi_plus, M_phi_minus)
    results = run_bass_kernel_spmd(nc, in_maps, core_ids=list(range(N_CORES)))

    out = np.empty((B, L, O), dtype=np.float32)
    for c in range(N_CORES):
        b, oq = divmod(c, NOQ)
        r = results.results[c]["out"]                   # [P, NB*OS]
        blk = r.reshape(P, NB, OS).transpose(1, 0, 2).reshape(L, OS)
        out[b, :, oq * OS:(oq + 1) * OS] = blk
    return out
